# revision 2
# baseline (speedup 1.0000x reference)
"""LlamaAttention (B=2,S=2048,H=4096, 32 q heads / 8 kv heads, RoPE, causal)
on 8 trn2 cores. Sharding: DP=2 over batch x TP=4 over heads.
Each core: 1 batch, 8 q heads, 2 kv heads. Host pre-transposes inputs to
[feature, token] layouts, casts to bf16; device computes partial output
(A_c @ Wo_c^T)^T in bf16; host sums 4 TP partials per batch in f32.

v2: software-pipelined instruction schedule keeps the PE (tensor engine)
continuously busy: score matmuls for block tb are interleaved at ~2:1 with
Q-projection matmuls for block tb+1 and Wo matmuls for block tb-1, so the
Act engine's exp throughput (2x slower per tile than PE) is hidden.
"""
import sys
if "/opt/trn_rl_repo" not in sys.path:
    sys.path.insert(0, "/opt/trn_rl_repo")

import numpy as np
import ml_dtypes

S = 2048
H = 4096
HD = 128
NHL = 8        # q heads per core
NKVL = 2       # kv heads per core
QF = NHL * HD  # 1024
KF = NKVL * HD  # 256
TB = 512       # token block
NTB = S // TB  # 4
KB = H // 128  # 32 contraction tiles for projections

_CACHE = {}
LAST = {}


def _build():
    if "nc" in _CACHE:
        return _CACHE["nc"]
    import concourse.bacc as bacc
    import concourse.mybir as mybir
    from concourse.tile import TileContext

    F32 = mybir.dt.float32
    BF16 = mybir.dt.bfloat16
    EXP = mybir.ActivationFunctionType.Exp
    SCALE = 1.0 / float(np.sqrt(HD))

    _ctr = [0]

    def _nm(p):
        _ctr[0] += 1
        return f"{p}{_ctr[0]}"

    nc = bacc.Bacc("TRN2", target_bir_lowering=False, debug=False, num_devices=8)
    xt = nc.declare_dram_parameter("xt", [H, S], BF16, isOutput=False)
    wqt = nc.declare_dram_parameter("wqt", [H, QF], BF16, isOutput=False)
    wkt = nc.declare_dram_parameter("wkt", [H, KF], BF16, isOutput=False)
    wvt = nc.declare_dram_parameter("wvt", [H, KF], BF16, isOutput=False)
    wot = nc.declare_dram_parameter("wot", [QF, H], BF16, isOutput=False)
    cs = nc.declare_dram_parameter("cs", [128, S], F32, isOutput=False)
    sn = nc.declare_dram_parameter("sn", [128, S], F32, isOutput=False)
    msk = nc.declare_dram_parameter("msk", [128, 4 * TB], BF16, isOutput=False)
    idn = nc.declare_dram_parameter("idn", [128, 128], BF16, isOutput=False)
    out_t = nc.declare_dram_parameter("out_t", [H, S], BF16, isOutput=True)

    wqt_r = wqt.rearrange("(kb p) m -> p kb m", p=128)
    wkt_r = wkt.rearrange("(kb p) m -> p kb m", p=128)
    wvt_r = wvt.rearrange("(kb p) m -> p kb m", p=128)
    wot_r = wot.rearrange("(kb p) m -> p kb m", p=128)

    from contextlib import ExitStack

    with ExitStack() as ctx:
        tc = ctx.enter_context(TileContext(nc))
        pc = ctx.enter_context(tc.tile_pool(name="const", bufs=1))
        px = ctx.enter_context(tc.tile_pool(name="xx", bufs=34))
        pwqk = ctx.enter_context(tc.tile_pool(name="wqk", bufs=3))
        pwv = ctx.enter_context(tc.tile_pool(name="wv", bufs=1))
        pwo = ctx.enter_context(tc.tile_pool(name="wo", bufs=4))
        pq = ctx.enter_context(tc.tile_pool(name="qt", bufs=17))
        pk = ctx.enter_context(tc.tile_pool(name="kt", bufs=2))
        pv = ctx.enter_context(tc.tile_pool(name="vv", bufs=16))
        pa = ctx.enter_context(tc.tile_pool(name="at", bufs=17))
        pp = ctx.enter_context(tc.tile_pool(name="pt", bufs=24))
        pasb = ctx.enter_context(tc.tile_pool(name="asb", bufs=4))
        pcs = ctx.enter_context(tc.tile_pool(name="csn", bufs=3))
        pr = ctx.enter_context(tc.tile_pool(name="rope", bufs=3))
        psmall = ctx.enter_context(tc.tile_pool(name="sm", bufs=8))
        pob = ctx.enter_context(tc.tile_pool(name="ob", bufs=4))
        psA = ctx.enter_context(tc.tile_pool(name="psA", bufs=3, space="PSUM"))
        psS = ctx.enter_context(tc.tile_pool(name="psS", bufs=3, space="PSUM"))
        psO = ctx.enter_context(tc.tile_pool(name="psO", bufs=2, space="PSUM"))
        if True:
            idn_sb = pc.tile([128, 128], BF16, tag="idn")
            nc.sync.dma_start(out=idn_sb[:], in_=idn[:])
            msk_sb = pc.tile([128, 4 * TB], BF16, tag="msk")
            nc.sync.dma_start(out=msk_sb[:], in_=msk[:])

            # persistent K^T [hd, S] per kv head; V_aug tiles [tok128,(kv,129)]
            kts = [pk.tile([128, S], BF16, tag="kt", name=f"ktp{i}")
                   for i in range(NKVL)]
            vts = []          # grows to 16 tiles [128, NKVL, 129]
            xts_map = {}      # tb -> list of 32 x tiles
            cssn_map = {}     # tb -> (cs_tile, sn_tile)
            qt_map = {}       # (tb, h) -> q tile [128, TB] bf16
            at_map = {}       # (tb, h) -> attn-out tile [128, TB] bf16

            def rope(dst, ps, cs_t, sn_t):
                tmp = pr.tile([128, TB], F32, tag="rsin", name=_nm("rsin"))
                nc.vector.tensor_mul(tmp[0:64, :], ps[64:128, :], sn_t[0:64, :])
                nc.vector.tensor_mul(tmp[64:128, :], ps[0:64, :], sn_t[64:128, :])
                tmp2 = pr.tile([128, TB], F32, tag="rcos", name=_nm("rcos"))
                nc.vector.tensor_mul(tmp2[:], ps[:], cs_t[:])
                nc.vector.tensor_add(dst, tmp[:], tmp2[:])

            def load_x_chunk(tb, k0, k1):
                tsl = slice(tb * TB, (tb + 1) * TB)
                tiles = xts_map.setdefault(tb, [])
                for k in range(k0, k1):
                    t = px.tile([128, TB], BF16, tag="xx", name=_nm("xx"))
                    nc.sync.dma_start(out=t[:], in_=xt[k * 128:(k + 1) * 128, tsl])
                    tiles.append(t)

            def load_x(tb):
                load_x_chunk(tb, 0, KB)

            def load_x_split(tb):
                # prologue only: split x issue across SP and Act queues to
                # halve delivery latency while the PE has nothing else to do
                tsl = slice(tb * TB, (tb + 1) * TB)
                tiles = xts_map.setdefault(tb, [])
                for k in range(KB):
                    t = px.tile([128, TB], BF16, tag="xx", name=_nm("xx"))
                    eng = nc.sync if k % 2 == 0 else nc.scalar
                    eng.dma_start(out=t[:], in_=xt[k * 128:(k + 1) * 128, tsl])
                    tiles.append(t)

            def load_cssn(tb):
                tsl = slice(tb * TB, (tb + 1) * TB)
                cs_t = pcs.tile([128, TB], F32, tag="cs", name=_nm("cs"))
                nc.sync.dma_start(out=cs_t[:], in_=cs[:, tsl])
                sn_t = pcs.tile([128, TB], F32, tag="sn", name=_nm("sn"))
                nc.sync.dma_start(out=sn_t[:], in_=sn[:, tsl])
                cssn_map[tb] = (cs_t, sn_t)

            def make_qproj(tbn, m):
                """Generator: Q projection chain for block tbn, head m.
                Yields once per PE matmul; finalizes with rope on DVE."""
                strip = pwqk.tile([128, KB, 128], BF16, tag="wqk", name=_nm("wqk"))
                nc.sync.dma_start(out=strip[:], in_=wqt_r[:, :, m * 128:(m + 1) * 128])
                ps = psA.tile([128, TB], F32, tag="A", name=_nm("psa"))
                xts = xts_map[tbn]
                cs_t, sn_t = cssn_map[tbn]

                def gen():
                    for k in range(KB):
                        nc.tensor.matmul(ps[:], strip[:, k, :], xts[k][:],
                                         start=(k == 0), stop=(k == KB - 1))
                        yield
                    qd = pq.tile([128, TB], BF16, tag="qt", name=_nm("qt"))
                    rope(qd[:], ps, cs_t, sn_t)
                    qt_map[(tbn, m)] = qd
                return gen()

            def load_kstrip(m):
                strip = pwqk.tile([128, KB, 128], BF16, tag="wqk", name=_nm("wqk"))
                nc.sync.dma_start(out=strip[:], in_=wkt_r[:, :, m * 128:(m + 1) * 128])
                return strip

            def kproj(tbn, m, strip=None):
                tsl = slice(tbn * TB, (tbn + 1) * TB)
                if strip is None:
                    strip = load_kstrip(m)
                ps = psA.tile([128, TB], F32, tag="A", name=_nm("psa"))
                xts = xts_map[tbn]
                cs_t, sn_t = cssn_map[tbn]
                for k in range(KB):
                    nc.tensor.matmul(ps[:], strip[:, k, :], xts[k][:],
                                     start=(k == 0), stop=(k == KB - 1))
                rope(kts[m][:, tsl], ps, cs_t, sn_t)

            def load_vstrip(tbn):
                vstrip = pwv.tile([128, KB, KF], BF16, tag="wv", name=_nm("wv"))
                nc.sync.dma_start(out=vstrip[:], in_=wvt_r[:, :, :])
                return vstrip

            def vproj(tbn, vstrip):
                xts = xts_map[tbn]
                for t in range(4):
                    ps = psA.tile([128, TB], F32, tag="A", name=_nm("psa"))
                    for k in range(KB):
                        nc.tensor.matmul(ps[:, 0:KF], xts[k][:, t * 128:(t + 1) * 128],
                                         vstrip[:, k, :], start=(k == 0),
                                         stop=(k == KB - 1))
                    vt = pv.tile([128, NKVL, 129], BF16, tag="vv", name=_nm("vv"))
                    for kv in range(NKVL):
                        nc.vector.tensor_copy(vt[:, kv, 0:128],
                                              ps[:, kv * 128:(kv + 1) * 128])
                    nc.vector.memset(vt[:, :, 128:129], 1.0)
                    vts.append(vt)

            def make_wo(tbs, of):
                """Generator: one Wo output-feature chain for block tbs."""
                strip = pwo.tile([128, NHL, 128], BF16, tag="wo", name=_nm("wo"))
                nc.sync.dma_start(out=strip[:], in_=wot_r[:, :, of * 128:(of + 1) * 128])
                ps = psA.tile([128, TB], F32, tag="A", name=_nm("psa"))

                def gen():
                    for hf in range(NHL):
                        nc.tensor.matmul(ps[:], strip[:, hf, :], at_map[(tbs, hf)][:],
                                         start=(hf == 0), stop=(hf == NHL - 1))
                        yield
                    ob = pob.tile([128, TB], BF16, tag="ob", name=_nm("ob"))
                    nc.vector.tensor_copy(ob[:], ps[:])
                    nc.sync.dma_start(
                        out=out_t[of * 128:(of + 1) * 128, tbs * TB:(tbs + 1) * TB],
                        in_=ob[:])
                return gen()

            def pv_head(tb, h, pts):
                kv = h // 4
                tr = psS.tile([128, TB], BF16, tag="st", name=_nm("tr"))
                for j in range(4):
                    nk = 4 * tb + j + 1
                    o = psO.tile([128, 129], F32, tag="o", name=_nm("o"))
                    for kt in range(nk):
                        nc.tensor.matmul(o[:, :], pts[kt][:, j * 128:(j + 1) * 128],
                                         vts[kt][:, kv, :], start=(kt == 0),
                                         stop=(kt == nk - 1))
                    r = psmall.tile([128, 1], F32, tag="r", name=_nm("r"))
                    nc.vector.reciprocal(r[:], o[:, 128:129])
                    a_sb = pasb.tile([128, 128], BF16, tag="asb", name=_nm("asb"))
                    nc.vector.tensor_scalar_mul(a_sb[:], o[:, 0:128], r[:])
                    nc.tensor.transpose(tr[:, j * 128:(j + 1) * 128], a_sb[:],
                                        idn_sb[:])
                ad = pa.tile([128, TB], BF16, tag="at", name=_nm("at"))
                nc.scalar.copy(ad[:], tr[:])
                at_map[(tb, h)] = ad

            def attn_cell(tb, h, fillers):
                """Score sweep for (tb, h) with ~2 filler matmuls interleaved
                per score tile, then drain fillers, then PV for this head."""
                kv = h // 4
                nkt = 4 * tb + 4
                qd = qt_map[(tb, h)]
                pts = []

                def filler_iter():
                    for g in fillers:
                        yield from g
                fit = filler_iter()

                for kt in range(nkt):
                    st = psS.tile([128, TB], F32, tag="st", name=_nm("st"))
                    ptile = pp.tile([128, TB], BF16, tag="pt", name=_nm("pt"))
                    if kt >= 4 * tb:
                        # diagonal 512-block: keys in this 128-tile only see
                        # queries q >= jj*128; compute/exp just that span and
                        # apply the 128-wide triangle mask on its first chunk
                        jj = kt - 4 * tb
                        q0 = jj * 128
                        nc.tensor.matmul(st[:, q0:], kts[kv][:, kt * 128:(kt + 1) * 128],
                                         qd[:, q0:], start=True, stop=True)
                        nc.scalar.activation(ptile[:, q0:], st[:, q0:], EXP,
                                             bias=0.0, scale=SCALE)
                        nc.vector.tensor_mul(
                            ptile[:, q0:q0 + 128], ptile[:, q0:q0 + 128],
                            msk_sb[:, jj * TB + q0:jj * TB + q0 + 128])
                    else:
                        nc.tensor.matmul(st[:], kts[kv][:, kt * 128:(kt + 1) * 128],
                                         qd[:], start=True, stop=True)
                        nc.scalar.activation(ptile[:], st[:], EXP, bias=0.0,
                                             scale=SCALE)
                    pts.append(ptile)
                    if kt % 3 == 2:
                        for _ in range(8):
                            next(fit, None)
                for _ in fit:
                    pass
                pv_head(tb, h, pts)

            # ---- prologue: x(0); K/V/Q projections for block 0.
            # x(1) is emitted in chunks between the Q chains so the SP queue
            # never blocks on x-pool ring slots ahead of cell-0's strip DMAs.
            ks0 = load_kstrip(0)
            ks1 = load_kstrip(1)
            load_cssn(0)
            load_cssn(1)
            load_x(0)
            kproj(0, 0, ks0)
            kproj(0, 1, ks1)
            vstrip0 = load_vstrip(0)
            vproj(0, vstrip0)
            for h in range(NHL):
                for _ in make_qproj(0, h):
                    pass
                if h >= 4:
                    load_x_chunk(1, 8 * (h - 4), 8 * (h - 3))

            # ---- steady-state: attention(tb) with proj(tb+1) + Wo(tb-1)
            vstrip_next = None
            for tb in range(NTB):
                if tb + 1 < NTB:
                    if tb + 2 < NTB:
                        load_cssn(tb + 2)
                    vstrip_next = load_vstrip(tb + 1)
                    if tb >= 1:
                        load_x(tb + 1)
                for h in range(NHL):
                    fillers = []
                    if tb + 1 < NTB:
                        fillers.append(make_qproj(tb + 1, h))
                    if tb >= 1:
                        for of in range(4 * h, 4 * h + 4):
                            fillers.append(make_wo(tb - 1, of))
                    attn_cell(tb, h, fillers)
                if tb + 1 < NTB:
                    kproj(tb + 1, 0)
                    kproj(tb + 1, 1)
                    vproj(tb + 1, vstrip_next)
                    xts_map.pop(tb, None)

            # ---- epilogue: Wo for the last block
            for of in range(H // 128):
                for _ in make_wo(NTB - 1, of):
                    pass

    nc.compile()
    _CACHE["nc"] = nc
    return nc


def _prep(hidden_states, Wq, Wk, Wv, Wo, position_ids):
    bf16 = ml_dtypes.bfloat16

    inv = 1.0 / (10000.0 ** (np.arange(0, HD, 2, dtype=np.float64) / HD))  # [64]
    kk = np.arange(128)[:, None]
    qq = np.arange(TB)[None, :]
    mskc = np.concatenate([(qq >= kk + 128 * j) for j in range(4)], axis=1)
    mskc = mskc.astype(bf16)
    idnc = np.eye(128, dtype=np.float32).astype(bf16)

    in_maps = []
    for c in range(8):
        b, g = c // 4, c % 4
        xtn = np.ascontiguousarray(hidden_states[b].T).astype(bf16)
        wqtc = np.ascontiguousarray(Wq[QF * g:QF * (g + 1), :].T).astype(bf16)
        wktc = np.ascontiguousarray(Wk[KF * g:KF * (g + 1), :].T).astype(bf16)
        wvtc = np.ascontiguousarray(Wv[KF * g:KF * (g + 1), :].T).astype(bf16)
        wotc = np.ascontiguousarray(Wo[:, QF * g:QF * (g + 1)].T).astype(bf16)
        pos = position_ids[b].astype(np.float64)
        ang = inv[:, None] * pos[None, :]  # [64, S]
        cosf = np.concatenate([np.cos(ang), np.cos(ang)], 0).astype(np.float32)
        sinb = np.sin(ang)
        sinf = np.concatenate([-sinb, sinb], 0).astype(np.float32)
        in_maps.append(dict(xt=xtn, wqt=wqtc, wkt=wktc, wvt=wvtc, wot=wotc,
                            cs=cosf, sn=sinf, msk=mskc, idn=idnc))
    return in_maps


def kernel(hidden_states, Wq, Wk, Wv, Wo, position_ids):
    from concourse.bass_utils import run_bass_kernel_spmd

    hidden_states = np.asarray(hidden_states)
    Wq, Wk, Wv, Wo = (np.asarray(a) for a in (Wq, Wk, Wv, Wo))
    position_ids = np.asarray(position_ids)
    B = hidden_states.shape[0]

    nc = _build()
    in_maps = _prep(hidden_states, Wq, Wk, Wv, Wo, position_ids)
    res = run_bass_kernel_spmd(nc, in_maps, list(range(8)))
    LAST["exec_time_ns"] = getattr(res, "exec_time_ns", None)

    out = np.empty((B, S, H), np.float32)
    for b in range(B):
        acc = res.results[4 * b]["out_t"].astype(np.float32)
        for g in range(1, 4):
            acc = acc + res.results[4 * b + g]["out_t"].astype(np.float32)
        out[b] = acc.T
    return out


def time_exec(hidden_states, Wq, Wk, Wv, Wo, position_ids, iters=5):
    """Time the on-device execution with device-resident inputs (mimics
    bass2jax.run_bass_via_pjrt's 8-core shard_map path, minus H2D)."""
    import jax
    import jax.numpy as jnp
    from jax.sharding import Mesh, PartitionSpec, NamedSharding
    from jax.experimental.shard_map import shard_map
    import time as _time
    from concourse import bass2jax, mybir

    nc = _build()
    in_maps = _prep(np.asarray(hidden_states), np.asarray(Wq), np.asarray(Wk),
                    np.asarray(Wv), np.asarray(Wo), np.asarray(position_ids))
    n_cores = 8
    bass2jax.install_neuronx_cc_hook()
    partition_name = nc.partition_id_tensor.name if nc.partition_id_tensor else None
    in_names, out_names, out_avals = [], [], []
    zero_shapes = []
    for alloc in nc.m.functions[0].allocations:
        if not isinstance(alloc, mybir.MemoryLocationSet):
            continue
        name = alloc.memorylocations[0].name
        if alloc.kind == "ExternalInput":
            if name != partition_name:
                in_names.append(name)
        elif alloc.kind == "ExternalOutput":
            out_names.append(name)
            shape = tuple(alloc.tensor_shape)
            dtype = mybir.dt.np(alloc.dtype)
            out_avals.append(jax.core.ShapedArray(shape, dtype))
            zero_shapes.append((shape, dtype))
    n_params = len(in_names)
    all_names = list(in_names) + list(out_names)
    if partition_name is not None:
        all_names.append(partition_name)

    def _body(*args):
        operands = list(args)
        if partition_name is not None:
            operands.append(bass2jax.partition_id_tensor())
        outs = bass2jax._bass_exec_p.bind(
            *operands,
            out_avals=tuple(out_avals),
            in_names=tuple(all_names),
            out_names=tuple(out_names),
            lowering_input_output_aliases=(),
            sim_require_finite=True,
            sim_require_nnan=True,
            nc=nc,
        )
        return tuple(outs)

    devices = jax.devices()[:n_cores]
    mesh = Mesh(np.asarray(devices), ("core",))
    nouts = len(out_names)
    donate = tuple(range(n_params, n_params + nouts))
    sharded = jax.jit(
        shard_map(_body, mesh=mesh,
                  in_specs=(PartitionSpec("core"),) * (n_params + nouts),
                  out_specs=(PartitionSpec("core"),) * nouts, check_rep=False),
        donate_argnums=donate, keep_unused=True)
    sh = NamedSharding(mesh, PartitionSpec("core"))
    dev_in = [jax.device_put(
        np.concatenate([np.asarray(in_maps[c][nm]) for c in range(n_cores)], 0), sh)
        for nm in in_names]

    def batch(n):
        zsets = [[jnp.zeros((n_cores * s[0], *s[1:]), d, device=sh)
                  for (s, d) in zero_shapes] for _ in range(n)]
        for z in zsets:
            jax.block_until_ready(z)
        t0 = _time.perf_counter()
        outs = [sharded(*dev_in, *z) for z in zsets]
        jax.block_until_ready(outs)
        return _time.perf_counter() - t0

    batch(1)  # warm compile/dispatch
    t1 = min(batch(1) for _ in range(3))
    tn = min(batch(iters) for _ in range(3))
    per_exec = (tn - t1) / (iters - 1)
    return {"t1": t1, "tn": tn, "iters": iters, "per_exec_s": per_exec}


# revision 4
# speedup vs baseline: 1.3983x; 1.3983x over previous
"""LlamaAttention (B=2,S=2048,H=4096, 32 q heads / 8 kv heads, RoPE, causal)
on 8 trn2 cores. Sharding: DP=2 over batch x TP=4 over heads.
Each core: 1 batch, 8 q heads, 2 kv heads. Host pre-transposes inputs to
[feature, token] layouts, casts to bf16; device computes partial output
(A_c @ Wo_c^T)^T in bf16; host sums 4 TP partials per batch in f32.

v2: software-pipelined instruction schedule keeps the PE (tensor engine)
continuously busy: score matmuls for block tb are interleaved at ~2:1 with
Q-projection matmuls for block tb+1 and Wo matmuls for block tb-1, so the
Act engine's exp throughput (2x slower per tile than PE) is hidden.
"""
import sys
if "/opt/trn_rl_repo" not in sys.path:
    sys.path.insert(0, "/opt/trn_rl_repo")

import numpy as np
import ml_dtypes

S = 2048
H = 4096
HD = 128
NHL = 8        # q heads per core
NKVL = 2       # kv heads per core
QF = NHL * HD  # 1024
KF = NKVL * HD  # 256
TB = 512       # token block
NTB = S // TB  # 4
KB = H // 128  # 32 contraction tiles for projections

_CACHE = {}
LAST = {}


def _build():
    if "nc" in _CACHE:
        return _CACHE["nc"]
    import concourse.bacc as bacc
    import concourse.mybir as mybir
    from concourse.tile import TileContext

    F32 = mybir.dt.float32
    BF16 = mybir.dt.bfloat16
    EXP = mybir.ActivationFunctionType.Exp
    SCALE = 1.0 / float(np.sqrt(HD))

    _ctr = [0]

    def _nm(p):
        _ctr[0] += 1
        return f"{p}{_ctr[0]}"

    nc = bacc.Bacc("TRN2", target_bir_lowering=False, debug=False, num_devices=8)
    xt = nc.declare_dram_parameter("xt", [H, S], BF16, isOutput=False)
    wqt = nc.declare_dram_parameter("wqt", [H, QF], BF16, isOutput=False)
    wkt = nc.declare_dram_parameter("wkt", [H, KF], BF16, isOutput=False)
    wvt = nc.declare_dram_parameter("wvt", [H, KF], BF16, isOutput=False)
    wot = nc.declare_dram_parameter("wot", [QF, H], BF16, isOutput=False)
    cs = nc.declare_dram_parameter("cs", [128, S], F32, isOutput=False)
    sn = nc.declare_dram_parameter("sn", [128, S], F32, isOutput=False)
    msk = nc.declare_dram_parameter("msk", [128, 4 * TB], BF16, isOutput=False)
    idn = nc.declare_dram_parameter("idn", [128, 128], BF16, isOutput=False)
    out_t = nc.declare_dram_parameter("out_t", [H, S], BF16, isOutput=True)

    xt_r = xt.rearrange("(kb p) s -> p kb s", p=128)
    wqt_r = wqt.rearrange("(kb p) m -> p kb m", p=128)
    wkt_r = wkt.rearrange("(kb p) m -> p kb m", p=128)
    wvt_r = wvt.rearrange("(kb p) m -> p kb m", p=128)
    wot_r = wot.rearrange("(kb p) m -> p kb m", p=128)

    from contextlib import ExitStack

    with ExitStack() as ctx:
        tc = ctx.enter_context(TileContext(nc))
        pc = ctx.enter_context(tc.tile_pool(name="const", bufs=1))
        px = ctx.enter_context(tc.tile_pool(name="xx", bufs=9))
        pwqk = ctx.enter_context(tc.tile_pool(name="wqk", bufs=3))
        pwv = ctx.enter_context(tc.tile_pool(name="wv", bufs=1))
        pwo = ctx.enter_context(tc.tile_pool(name="wo", bufs=4))
        pq = ctx.enter_context(tc.tile_pool(name="qt", bufs=17))
        pk = ctx.enter_context(tc.tile_pool(name="kt", bufs=2))
        pv = ctx.enter_context(tc.tile_pool(name="vv", bufs=16))
        pa = ctx.enter_context(tc.tile_pool(name="at", bufs=17))
        pp = ctx.enter_context(tc.tile_pool(name="pt", bufs=24))
        pasb = ctx.enter_context(tc.tile_pool(name="asb", bufs=4))
        pcs = ctx.enter_context(tc.tile_pool(name="csn", bufs=3))
        pr = ctx.enter_context(tc.tile_pool(name="rope", bufs=3))
        psmall = ctx.enter_context(tc.tile_pool(name="sm", bufs=8))
        pob = ctx.enter_context(tc.tile_pool(name="ob", bufs=4))
        psA = ctx.enter_context(tc.tile_pool(name="psA", bufs=3, space="PSUM"))
        psS = ctx.enter_context(tc.tile_pool(name="psS", bufs=3, space="PSUM"))
        psO = ctx.enter_context(tc.tile_pool(name="psO", bufs=2, space="PSUM"))
        if True:
            idn_sb = pc.tile([128, 128], BF16, tag="idn")
            nc.sync.dma_start(out=idn_sb[:], in_=idn[:])
            msk_sb = pc.tile([128, 4 * TB], BF16, tag="msk")
            nc.sync.dma_start(out=msk_sb[:], in_=msk[:])

            # persistent K^T [hd, S] per kv head; V_aug tiles [tok128,(kv,129)]
            kts = [pk.tile([128, S], BF16, tag="kt", name=f"ktp{i}")
                   for i in range(NKVL)]
            vts = []          # grows to 16 tiles [128, NKVL, 129]
            xts_map = {}      # tb -> list of 32 x tiles
            cssn_map = {}     # tb -> (cs_tile, sn_tile)
            qt_map = {}       # (tb, h) -> q tile [128, TB] bf16
            at_map = {}       # (tb, h) -> attn-out tile [128, TB] bf16

            def rope(dst, ps, cs_t, sn_t):
                tmp = pr.tile([128, TB], F32, tag="rsin", name=_nm("rsin"))
                nc.vector.tensor_mul(tmp[0:64, :], ps[64:128, :], sn_t[0:64, :])
                nc.vector.tensor_mul(tmp[64:128, :], ps[0:64, :], sn_t[64:128, :])
                tmp2 = pr.tile([128, TB], F32, tag="rcos", name=_nm("rcos"))
                nc.vector.tensor_mul(tmp2[:], ps[:], cs_t[:])
                nc.vector.tensor_add(dst, tmp[:], tmp2[:])

            def load_x_chunk(tb, g0, g1):
                # packed: one DMA per 4 contraction chunks [128, 4, TB]
                tsl = slice(tb * TB, (tb + 1) * TB)
                tiles = xts_map.setdefault(tb, [])
                for g in range(g0, g1):
                    t = px.tile([128, 4, TB], BF16, tag="xx", name=_nm("xx"))
                    nc.sync.dma_start(out=t[:], in_=xt_r[:, 4 * g:4 * g + 4, tsl])
                    tiles.append(t)

            def load_x(tb):
                load_x_chunk(tb, 0, KB // 4)

            def xck(tbn, k):
                return xts_map[tbn][k // 4][:, k % 4, :]

            def load_cssn(tb):
                tsl = slice(tb * TB, (tb + 1) * TB)
                cs_t = pcs.tile([128, TB], F32, tag="cs", name=_nm("cs"))
                nc.sync.dma_start(out=cs_t[:], in_=cs[:, tsl])
                sn_t = pcs.tile([128, TB], F32, tag="sn", name=_nm("sn"))
                nc.sync.dma_start(out=sn_t[:], in_=sn[:, tsl])
                cssn_map[tb] = (cs_t, sn_t)

            def make_qproj(tbn, m):
                """Generator: Q projection chain for block tbn, head m.
                Yields once per PE matmul; finalizes with rope on DVE."""
                strip = pwqk.tile([128, KB, 128], BF16, tag="wqk", name=_nm("wqk"))
                nc.sync.dma_start(out=strip[:], in_=wqt_r[:, :, m * 128:(m + 1) * 128])
                ps = psA.tile([128, TB], F32, tag="A", name=_nm("psa"))
                cs_t, sn_t = cssn_map[tbn]

                def gen():
                    for k in range(KB):
                        nc.tensor.matmul(ps[:], strip[:, k, :], xck(tbn, k),
                                         start=(k == 0), stop=(k == KB - 1))
                        yield
                    qd = pq.tile([128, TB], BF16, tag="qt", name=_nm("qt"))
                    rope(qd[:], ps, cs_t, sn_t)
                    qt_map[(tbn, m)] = qd
                return gen()

            def load_kstrip(m):
                strip = pwqk.tile([128, KB, 128], BF16, tag="wqk", name=_nm("wqk"))
                nc.sync.dma_start(out=strip[:], in_=wkt_r[:, :, m * 128:(m + 1) * 128])
                return strip

            def kproj(tbn, m, strip=None):
                tsl = slice(tbn * TB, (tbn + 1) * TB)
                if strip is None:
                    strip = load_kstrip(m)
                ps = psA.tile([128, TB], F32, tag="A", name=_nm("psa"))
                cs_t, sn_t = cssn_map[tbn]
                for k in range(KB):
                    nc.tensor.matmul(ps[:], strip[:, k, :], xck(tbn, k),
                                     start=(k == 0), stop=(k == KB - 1))
                rope(kts[m][:, tsl], ps, cs_t, sn_t)

            def load_vstrip(tbn):
                vstrip = pwv.tile([128, KB, KF], BF16, tag="wv", name=_nm("wv"))
                nc.sync.dma_start(out=vstrip[:, 0:KB // 2, :],
                                  in_=wvt_r[:, 0:KB // 2, :])
                nc.sync.dma_start(out=vstrip[:, KB // 2:, :],
                                  in_=wvt_r[:, KB // 2:, :])
                return vstrip

            def vproj(tbn, vstrip):
                for t in range(4):
                    ps = psA.tile([128, TB], F32, tag="A", name=_nm("psa"))
                    for k in range(KB):
                        nc.tensor.matmul(ps[:, 0:KF],
                                         xck(tbn, k)[:, t * 128:(t + 1) * 128],
                                         vstrip[:, k, :], start=(k == 0),
                                         stop=(k == KB - 1))
                    vt = pv.tile([128, NKVL, 129], BF16, tag="vv", name=_nm("vv"))
                    for kv in range(NKVL):
                        nc.vector.tensor_copy(vt[:, kv, 0:128],
                                              ps[:, kv * 128:(kv + 1) * 128])
                    nc.vector.memset(vt[:, :, 128:129], 1.0)
                    vts.append(vt)

            def make_wo(tbs, of):
                """Generator: one Wo output-feature chain for block tbs."""
                strip = pwo.tile([128, NHL, 128], BF16, tag="wo", name=_nm("wo"))
                nc.sync.dma_start(out=strip[:], in_=wot_r[:, :, of * 128:(of + 1) * 128])
                ps = psA.tile([128, TB], F32, tag="A", name=_nm("psa"))

                def gen():
                    for hf in range(NHL):
                        nc.tensor.matmul(ps[:], strip[:, hf, :], at_map[(tbs, hf)][:],
                                         start=(hf == 0), stop=(hf == NHL - 1))
                        yield
                    ob = pob.tile([128, TB], BF16, tag="ob", name=_nm("ob"))
                    nc.vector.tensor_copy(ob[:], ps[:])
                    nc.sync.dma_start(
                        out=out_t[of * 128:(of + 1) * 128, tbs * TB:(tbs + 1) * TB],
                        in_=ob[:])
                return gen()

            def pv_head(tb, h, pts):
                kv = h // 4
                tr = psS.tile([128, TB], BF16, tag="st", name=_nm("tr"))
                for j in range(4):
                    nk = 4 * tb + j + 1
                    o = psO.tile([128, 129], F32, tag="o", name=_nm("o"))
                    for kt in range(nk):
                        nc.tensor.matmul(o[:, :], pts[kt][:, j * 128:(j + 1) * 128],
                                         vts[kt][:, kv, :], start=(kt == 0),
                                         stop=(kt == nk - 1))
                    r = psmall.tile([128, 1], F32, tag="r", name=_nm("r"))
                    nc.vector.reciprocal(r[:], o[:, 128:129])
                    a_sb = pasb.tile([128, 128], BF16, tag="asb", name=_nm("asb"))
                    nc.vector.tensor_scalar_mul(a_sb[:], o[:, 0:128], r[:])
                    nc.tensor.transpose(tr[:, j * 128:(j + 1) * 128], a_sb[:],
                                        idn_sb[:])
                ad = pa.tile([128, TB], BF16, tag="at", name=_nm("at"))
                nc.scalar.copy(ad[:], tr[:])
                at_map[(tb, h)] = ad

            def attn_cell(tb, h, fillers):
                """Score sweep for (tb, h) with ~2 filler matmuls interleaved
                per score tile, then drain fillers, then PV for this head."""
                kv = h // 4
                nkt = 4 * tb + 4
                qd = qt_map[(tb, h)]
                pts = []

                def filler_iter():
                    for g in fillers:
                        yield from g
                fit = filler_iter()

                for kt in range(nkt):
                    st = psS.tile([128, TB], F32, tag="st", name=_nm("st"))
                    ptile = pp.tile([128, TB], BF16, tag="pt", name=_nm("pt"))
                    if kt >= 4 * tb:
                        # diagonal 512-block: keys in this 128-tile only see
                        # queries q >= jj*128; compute/exp just that span and
                        # apply the 128-wide triangle mask on its first chunk
                        jj = kt - 4 * tb
                        q0 = jj * 128
                        nc.tensor.matmul(st[:, q0:], kts[kv][:, kt * 128:(kt + 1) * 128],
                                         qd[:, q0:], start=True, stop=True)
                        nc.scalar.activation(ptile[:, q0:], st[:, q0:], EXP,
                                             bias=0.0, scale=SCALE)
                        nc.vector.tensor_mul(
                            ptile[:, q0:q0 + 128], ptile[:, q0:q0 + 128],
                            msk_sb[:, jj * TB + q0:jj * TB + q0 + 128])
                    else:
                        nc.tensor.matmul(st[:], kts[kv][:, kt * 128:(kt + 1) * 128],
                                         qd[:], start=True, stop=True)
                        nc.scalar.activation(ptile[:], st[:], EXP, bias=0.0,
                                             scale=SCALE)
                    pts.append(ptile)
                    if kt % 3 == 2:
                        for _ in range(8):
                            next(fit, None)
                for _ in fit:
                    pass
                pv_head(tb, h, pts)

            # ---- prologue: x(0); K/V/Q projections for block 0.
            # x(1) is emitted in chunks between the Q chains so the SP queue
            # never blocks on x-pool ring slots ahead of cell-0's strip DMAs.
            ks0 = load_kstrip(0)
            ks1 = load_kstrip(1)
            load_cssn(0)
            load_cssn(1)
            load_x(0)
            kproj(0, 0, ks0)
            kproj(0, 1, ks1)
            vstrip0 = load_vstrip(0)
            vproj(0, vstrip0)
            for h in range(NHL):
                for _ in make_qproj(0, h):
                    pass
                if h >= 4:
                    load_x_chunk(1, 2 * (h - 4), 2 * (h - 3))

            # ---- steady-state: attention(tb) with proj(tb+1) + Wo(tb-1)
            vstrip_next = None
            for tb in range(NTB):
                if tb + 1 < NTB:
                    if tb + 2 < NTB:
                        load_cssn(tb + 2)
                    vstrip_next = load_vstrip(tb + 1)
                    if tb >= 1:
                        load_x(tb + 1)
                for h in range(NHL):
                    fillers = []
                    if tb + 1 < NTB:
                        fillers.append(make_qproj(tb + 1, h))
                    if tb >= 1:
                        for of in range(4 * h, 4 * h + 4):
                            fillers.append(make_wo(tb - 1, of))
                    attn_cell(tb, h, fillers)
                if tb + 1 < NTB:
                    kproj(tb + 1, 0)
                    kproj(tb + 1, 1)
                    vproj(tb + 1, vstrip_next)
                    xts_map.pop(tb, None)

            # ---- epilogue: Wo for the last block
            for of in range(H // 128):
                for _ in make_wo(NTB - 1, of):
                    pass

    nc.compile()
    _CACHE["nc"] = nc
    return nc


def _prep(hidden_states, Wq, Wk, Wv, Wo, position_ids):
    bf16 = ml_dtypes.bfloat16

    inv = 1.0 / (10000.0 ** (np.arange(0, HD, 2, dtype=np.float64) / HD))  # [64]
    kk = np.arange(128)[:, None]
    qq = np.arange(TB)[None, :]
    mskc = np.concatenate([(qq >= kk + 128 * j) for j in range(4)], axis=1)
    mskc = mskc.astype(bf16)
    idnc = np.eye(128, dtype=np.float32).astype(bf16)

    in_maps = []
    for c in range(8):
        b, g = c // 4, c % 4
        xtn = np.ascontiguousarray(hidden_states[b].T).astype(bf16)
        wqtc = np.ascontiguousarray(Wq[QF * g:QF * (g + 1), :].T).astype(bf16)
        wktc = np.ascontiguousarray(Wk[KF * g:KF * (g + 1), :].T).astype(bf16)
        wvtc = np.ascontiguousarray(Wv[KF * g:KF * (g + 1), :].T).astype(bf16)
        wotc = np.ascontiguousarray(Wo[:, QF * g:QF * (g + 1)].T).astype(bf16)
        pos = position_ids[b].astype(np.float64)
        ang = inv[:, None] * pos[None, :]  # [64, S]
        cosf = np.concatenate([np.cos(ang), np.cos(ang)], 0).astype(np.float32)
        sinb = np.sin(ang)
        sinf = np.concatenate([-sinb, sinb], 0).astype(np.float32)
        in_maps.append(dict(xt=xtn, wqt=wqtc, wkt=wktc, wvt=wvtc, wot=wotc,
                            cs=cosf, sn=sinf, msk=mskc, idn=idnc))
    return in_maps


def kernel(hidden_states, Wq, Wk, Wv, Wo, position_ids):
    from concourse.bass_utils import run_bass_kernel_spmd

    hidden_states = np.asarray(hidden_states)
    Wq, Wk, Wv, Wo = (np.asarray(a) for a in (Wq, Wk, Wv, Wo))
    position_ids = np.asarray(position_ids)
    B = hidden_states.shape[0]

    nc = _build()
    in_maps = _prep(hidden_states, Wq, Wk, Wv, Wo, position_ids)
    res = run_bass_kernel_spmd(nc, in_maps, list(range(8)))
    LAST["exec_time_ns"] = getattr(res, "exec_time_ns", None)

    out = np.empty((B, S, H), np.float32)
    for b in range(B):
        acc = res.results[4 * b]["out_t"].astype(np.float32)
        for g in range(1, 4):
            acc = acc + res.results[4 * b + g]["out_t"].astype(np.float32)
        out[b] = acc.T
    return out


def time_exec(hidden_states, Wq, Wk, Wv, Wo, position_ids, iters=5):
    """Time the on-device execution with device-resident inputs (mimics
    bass2jax.run_bass_via_pjrt's 8-core shard_map path, minus H2D)."""
    import jax
    import jax.numpy as jnp
    from jax.sharding import Mesh, PartitionSpec, NamedSharding
    from jax.experimental.shard_map import shard_map
    import time as _time
    from concourse import bass2jax, mybir

    nc = _build()
    in_maps = _prep(np.asarray(hidden_states), np.asarray(Wq), np.asarray(Wk),
                    np.asarray(Wv), np.asarray(Wo), np.asarray(position_ids))
    n_cores = 8
    bass2jax.install_neuronx_cc_hook()
    partition_name = nc.partition_id_tensor.name if nc.partition_id_tensor else None
    in_names, out_names, out_avals = [], [], []
    zero_shapes = []
    for alloc in nc.m.functions[0].allocations:
        if not isinstance(alloc, mybir.MemoryLocationSet):
            continue
        name = alloc.memorylocations[0].name
        if alloc.kind == "ExternalInput":
            if name != partition_name:
                in_names.append(name)
        elif alloc.kind == "ExternalOutput":
            out_names.append(name)
            shape = tuple(alloc.tensor_shape)
            dtype = mybir.dt.np(alloc.dtype)
            out_avals.append(jax.core.ShapedArray(shape, dtype))
            zero_shapes.append((shape, dtype))
    n_params = len(in_names)
    all_names = list(in_names) + list(out_names)
    if partition_name is not None:
        all_names.append(partition_name)

    def _body(*args):
        operands = list(args)
        if partition_name is not None:
            operands.append(bass2jax.partition_id_tensor())
        outs = bass2jax._bass_exec_p.bind(
            *operands,
            out_avals=tuple(out_avals),
            in_names=tuple(all_names),
            out_names=tuple(out_names),
            lowering_input_output_aliases=(),
            sim_require_finite=True,
            sim_require_nnan=True,
            nc=nc,
        )
        return tuple(outs)

    devices = jax.devices()[:n_cores]
    mesh = Mesh(np.asarray(devices), ("core",))
    nouts = len(out_names)
    donate = tuple(range(n_params, n_params + nouts))
    sharded = jax.jit(
        shard_map(_body, mesh=mesh,
                  in_specs=(PartitionSpec("core"),) * (n_params + nouts),
                  out_specs=(PartitionSpec("core"),) * nouts, check_rep=False),
        donate_argnums=donate, keep_unused=True)
    sh = NamedSharding(mesh, PartitionSpec("core"))
    dev_in = [jax.device_put(
        np.concatenate([np.asarray(in_maps[c][nm]) for c in range(n_cores)], 0), sh)
        for nm in in_names]

    def batch(n):
        zsets = [[jnp.zeros((n_cores * s[0], *s[1:]), d, device=sh)
                  for (s, d) in zero_shapes] for _ in range(n)]
        for z in zsets:
            jax.block_until_ready(z)
        t0 = _time.perf_counter()
        outs = [sharded(*dev_in, *z) for z in zsets]
        jax.block_until_ready(outs)
        return _time.perf_counter() - t0

    batch(1)  # warm compile/dispatch
    t1 = min(batch(1) for _ in range(3))
    tn = min(batch(iters) for _ in range(3))
    per_exec = (tn - t1) / (iters - 1)
    return {"t1": t1, "tn": tn, "iters": iters, "per_exec_s": per_exec}
